# revision 1
# baseline (speedup 1.0000x reference)
"""Causal self-attention TRN2 kernel.

Problem: B=4, L=2048, D=768, H=6 heads, head_dim=128, fp32, causal mask
tril(k=1) (each query row q attends to keys k <= q+1).

Sharding: 8 cores = 4 batches x 2 head-groups (3 heads each).
Each core computes, for its batch b and heads [3g, 3g+3):
    Q = x_b @ Wq[:, cols] + bq[cols]   (and K, V likewise)
    per head: S^T = K @ Q^T (scaled), P = exp(S) masked, O = P@V / rowsum
    y_core = (O_heads @ Wo[rows, :])^T          -> [768, 2048] partial
Host: out[b] = (y[2b] + y[2b+1])^T + bo + bv @ Wo   (attn rows sum to 1,
so the V bias contributes exactly bv @ Wo_rows to every output row).

Layout trick: everything is kept transposed (feature dim on partitions) so
every matmul has a 512-wide moving operand and can run at full PE rate in
float32r (x itself arrives host-transposed, so no on-chip transposes at
all). Softmax runs without max-subtraction (logits are O(1) here), with
row sums computed by a ones-vector matmul in the same transposed layout,
then broadcast via a rank-1 matmul for the normalization multiply.
Projections run one 512-column chunk ahead of attention because the
tril(k=1) mask lets each query attend one token into the future.
Diagonal-band blocks restrict S/exp/mask/PV/rowsum to the valid column
range (everything below 128j-2 is structurally masked), cutting ~12% of
PE work.
"""

import math
from contextlib import ExitStack

import numpy as np

import concourse.tile as tile
from concourse import bacc, mybir
from concourse.bass_utils import run_bass_kernel_spmd

F32 = mybir.dt.float32
F32R = mybir.dt.float32r
AF = mybir.ActivationFunctionType

B, L, D, H = 4, 2048, 768, 6
HD = 128           # head dim
HPC = 3            # heads per core
DH = HPC * HD      # 384: per-core projection width
NCORES = 8
P = 128
CHUNK = 512        # q-chunk width (moving-operand size)
NCHUNK = L // CHUNK
LT = L // P        # 16 L-tiles
DT = D // P        # 6 d-tiles
SCALE = 1.0 / math.sqrt(HD)

_cache = {}


def build_nc(mm_fast=True, enable_asserts=False, reps=1,
             mm1_bufs=3, aux_bufs=1, pb_in_mm1=False, den_on_dve=False,
             est_bufs=6, attn_mode="seq", mask_gpsimd=False, vt_on_act=False,
             fused_denb=True, qk_on_dve=False, yst_on_act=False,
             attn_order="pipeline", qt_bufs=2, denom_mode="mm"):
    nc = bacc.Bacc(
        "TRN2",
        target_bir_lowering=False,
        debug=False,
        enable_asserts=enable_asserts,
        num_devices=NCORES,
    )
    x_d = nc.dram_tensor("x", [D, L], F32, kind="ExternalInput").ap()
    wq_d = nc.dram_tensor("wq", [D, DH], F32, kind="ExternalInput").ap()
    wk_d = nc.dram_tensor("wk", [D, DH], F32, kind="ExternalInput").ap()
    wv_d = nc.dram_tensor("wv", [D, DH], F32, kind="ExternalInput").ap()
    wo_d = nc.dram_tensor("wo", [DH, D], F32, kind="ExternalInput").ap()
    bq_d = nc.dram_tensor("bq", [DH], F32, kind="ExternalInput").ap()
    bk_d = nc.dram_tensor("bk", [DH], F32, kind="ExternalInput").ap()
    y_d = nc.dram_tensor("y", [D, L], F32, kind="ExternalOutput").ap()

    MMDT = F32R if mm_fast else F32
    cast = lambda ap: ap  # noqa: E731

    with tile.TileContext(nc) as tc, ExitStack() as ctx:
        const = ctx.enter_context(tc.tile_pool(name="const", bufs=1))
        wpool = ctx.enter_context(tc.tile_pool(name="wts", bufs=1))
        kvpool = ctx.enter_context(tc.tile_pool(name="kv", bufs=1))
        xtpool = ctx.enter_context(tc.tile_pool(name="xt", bufs=2))
        qpool = ctx.enter_context(tc.tile_pool(name="qt", bufs=qt_bufs))
        estpool = ctx.enter_context(tc.tile_pool(name="est", bufs=est_bufs))
        smpool = ctx.enter_context(tc.tile_pool(name="sm", bufs=2))
        opool = ctx.enter_context(tc.tile_pool(name="ot", bufs=2))
        espool = ctx.enter_context(tc.tile_pool(name="esum", bufs=2))
        ypool = ctx.enter_context(tc.tile_pool(name="yst", bufs=3))
        ps_mm = ctx.enter_context(tc.tile_pool(name="psmm", bufs=mm1_bufs, space="PSUM"))
        ps_acc = ctx.enter_context(tc.tile_pool(name="psacc", bufs=2, space="PSUM"))
        ps_aux = ctx.enter_context(tc.tile_pool(name="psaux", bufs=aux_bufs, space="PSUM"))

        ones_f32 = const.tile([P, 1], F32, tag="ones_f32", name="ones_f32")
        nc.vector.memset(ones_f32[:], 1.0)
        ones_col = const.tile([P, 1], MMDT, tag="ones_col", name="ones_col")
        nc.scalar.copy(ones_col[:], ones_f32[:])
        ones_mf = const.tile([P, P], F32, tag="ones_mf", name="ones_mf")
        nc.vector.memset(ones_mf[:], 1.0)
        ones_mat = const.tile([P, P], MMDT, tag="ones_mat", name="ones_mat")
        nc.scalar.copy(ones_mat[:], ones_mf[:])
        ones_rf = const.tile([1, P], F32, tag="ones_rf", name="ones_rf")
        nc.vector.memset(ones_rf[:], 1.0)
        ones_row = const.tile([1, P], MMDT, tag="ones_row", name="ones_row")
        nc.scalar.copy(ones_row[:], ones_rf[:])
        # 5 diagonal-band masks (0/1), shared by all chunks/heads.
        # mask[j][kp, qq] = 1 iff kp - qq <= 1 - 128*j
        masks = []
        for j in range(5):
            mj = const.tile([P, CHUNK], F32, tag=f"mask{j}", name=f"mask{j}")
            nc.gpsimd.memset(mj[:], 1.0)
            # keep (mask=1) where kp - qq <= 1 - 128*j, i.e. qq - kp + (1-128j) >= 0
            nc.gpsimd.affine_select(
                out=mj[:],
                in_=mj[:],
                pattern=[[1, CHUNK]],
                compare_op=mybir.AluOpType.is_ge,
                fill=0.0,
                base=1 - 128 * j,
                channel_multiplier=-1,
            )
            masks.append(mj)

        def make_xT(c):
            # xT columns: block d lives at [d*CHUNK, (d+1)*CHUNK)
            xT = xtpool.tile([P, DT * CHUNK], MMDT, tag="xT", name="xT")
            for d in range(DT):
                nc.sync.dma_start(
                    out=xT[:, d * CHUNK:(d + 1) * CHUNK],
                    in_=x_d[d * P:(d + 1) * P,
                            c * CHUNK:(c + 1) * CHUNK].bitcast(MMDT),
                )
            return xT

        # DMA issue order: wq tiles and chunk-0 x columns first so the first
        # projection matmuls can start early; wk/wv next; wo/biases later.
        wq = []
        for d in range(DT):
            wq_t = wpool.tile([P, DH], MMDT, tag=f"wq{d}", name=f"wq{d}")
            nc.sync.dma_start(out=wq_t[:], in_=wq_d[d * P:(d + 1) * P, :].bitcast(MMDT))
            wq.append(wq_t)
        xT0 = make_xT(0)
        wk = []
        wv = []
        for d in range(DT):
            wk_t = wpool.tile([P, DH], MMDT, tag=f"wk{d}", name=f"wk{d}")
            nc.sync.dma_start(out=wk_t[:], in_=wk_d[d * P:(d + 1) * P, :].bitcast(MMDT))
            wk.append(wk_t)
            wv_t = wpool.tile([P, DH], MMDT, tag=f"wv{d}", name=f"wv{d}")
            nc.sync.dma_start(out=wv_t[:], in_=wv_d[d * P:(d + 1) * P, :].bitcast(MMDT))
            wv.append(wv_t)
        bq_t = []
        bk_t = []
        for h in range(HPC):
            bq_h = wpool.tile([P, 1], F32, tag=f"bq{h}", name=f"bq{h}")
            nc.sync.dma_start(
                out=bq_h[:], in_=bq_d[h * P:(h + 1) * P].rearrange("(p o) -> p o", o=1)
            )
            bq_t.append(bq_h)
            bk_h = wpool.tile([P, 1], F32, tag=f"bk{h}", name=f"bk{h}")
            nc.sync.dma_start(
                out=bk_h[:], in_=bk_d[h * P:(h + 1) * P].rearrange("(p o) -> p o", o=1)
            )
            bk_t.append(bk_h)
        wo = []
        for h in range(HPC):
            wo_t = wpool.tile([P, D], MMDT, tag=f"wo{h}", name=f"wo{h}")
            nc.sync.dma_start(out=wo_t[:], in_=wo_d[h * P:(h + 1) * P, :].bitcast(MMDT))
            wo.append(wo_t)

        # K^T per head [hd=128, L]; V per L-tile [kpos=128, 3*hd]
        kT = [kvpool.tile([P, L], MMDT, tag=f"kT{h}", name=f"kT{h}") for h in range(HPC)]
        vt = [kvpool.tile([P, DH], MMDT, tag=f"v{t}", name=f"v{t}") for t in range(LT)]

        def proj_chunk(c, xT=None):
            # ---- x^T columns for this chunk (x arrives host-transposed) ----
            if xT is None:
                xT = make_xT(c)

            # ---- Q^T, K^T projections for this chunk ----
            qT = [qpool.tile([P, CHUNK], MMDT, tag=f"qT{h}", name=f"qT{h}")
                  for h in range(HPC)]
            for h in range(HPC):
                pq = ps_acc.tile([P, CHUNK], F32, tag="acc", name="acc")
                for d in range(DT):
                    nc.tensor.matmul(
                        pq[:],
                        cast(wq[d][:, h * P:(h + 1) * P]),
                        cast(xT[:, d * CHUNK:(d + 1) * CHUNK]),
                        start=(d == 0),
                        stop=(d == DT - 1),
                    )
                if qk_on_dve:
                    nc.vector.tensor_scalar_add(qT[h][:], pq[:], bq_t[h][:])
                else:
                    nc.scalar.activation(qT[h][:], pq[:], AF.Identity,
                                         bias=bq_t[h][:])
                pk = ps_acc.tile([P, CHUNK], F32, tag="acc", name="acc")
                for d in range(DT):
                    nc.tensor.matmul(
                        pk[:],
                        cast(wk[d][:, h * P:(h + 1) * P]),
                        cast(xT[:, d * CHUNK:(d + 1) * CHUNK]),
                        start=(d == 0),
                        stop=(d == DT - 1),
                    )
                if qk_on_dve:
                    nc.vector.tensor_scalar_add(
                        kT[h][:, c * CHUNK:(c + 1) * CHUNK], pk[:], bk_t[h][:]
                    )
                else:
                    nc.scalar.activation(
                        kT[h][:, c * CHUNK:(c + 1) * CHUNK], pk[:], AF.Identity,
                        bias=bk_t[h][:],
                    )

            # ---- V projection (natural layout) ----
            for i in range(CHUNK // P):
                t = c * (CHUNK // P) + i
                pv = ps_acc.tile([P, DH], F32, tag="acc", name="acc")
                for d in range(DT):
                    nc.tensor.matmul(
                        pv[:],
                        cast(xT[:, d * CHUNK + i * P: d * CHUNK + (i + 1) * P]),
                        cast(wv[d][:]),
                        start=(d == 0),
                        stop=(d == DT - 1),
                    )
                if vt_on_act:
                    nc.scalar.copy(vt[t][:], pv[:])
                else:
                    nc.vector.tensor_copy(vt[t][:], pv[:])
            return qT

        def attn_chunk_ileave(c, qT):
            # ---- attention, 3 heads interleaved per kb block ----
            # Denominators for all heads pack into one PSUM bank (rows
            # 0/32/64 -- tile_position requires 32-aligned output rows).
            # A single start=True (h0,kb0) clears the bank; the other heads'
            # first writes overwrite via the has_written bits.
            KB = 4 * c + 5 if c < NCHUNK - 1 else LT
            oTn = [opool.tile([P, CHUNK], MMDT, tag=f"oT{h}", name=f"oT{h}")
                   for h in range(HPC)]
            po = [ps_acc.tile([P, CHUNK], F32, tag=f"pv{h}", name=f"pv{h}", bufs=1)
                  for h in range(HPC)]
            pdall = ps_aux.tile([P, CHUNK], F32, tag="aux", name="aux", bufs=1)
            nc.vector.memset(pdall[:], 0.0)
            for kb in range(KB):
                j = kb - 4 * c
                for h in range(HPC):
                    pst = ps_mm.tile([P, CHUNK], F32, tag="mm1", name="mm1")
                    nc.tensor.matmul(
                        pst[:],
                        cast(kT[h][:, kb * P:(kb + 1) * P]),
                        cast(qT[h][:]),
                        start=True,
                        stop=True,
                    )
                    est = estpool.tile([P, CHUNK], MMDT, tag="est", name="est")
                    nc.scalar.activation(est[:], pst[:], AF.Exp, scale=SCALE)
                    if j >= 0:
                        nc.vector.tensor_mul(est[:], est[:], masks[j][:])
                    nc.tensor.matmul(
                        po[h][:],
                        cast(vt[kb][:, h * P:(h + 1) * P]),
                        cast(est[:]),
                        start=(kb == 0),
                        stop=(kb == KB - 1),
                    )
                    # All three heads' row sums accumulate into one PSUM bank
                    # (rows 0/32/64). The bank is DVE-memset to zero up front,
                    # so plain accumulation (never start=True) is correct on
                    # both hardware and sim regardless of has_written state.
                    nc.tensor.matmul(
                        pdall[32 * h:32 * h + 1, :],
                        cast(ones_col[:]),
                        cast(est[:]),
                        start=False,
                        stop=(kb == KB - 1 and h == HPC - 1),
                        skip_group_check=True,
                    )
            for h in range(HPC):
                den_sb = smpool.tile([1, CHUNK], MMDT, tag=f"den{h}", name=f"den{h}")
                nc.scalar.copy(den_sb[:], pdall[32 * h:32 * h + 1, :])
                pb = ps_mm.tile([P, CHUNK], F32, tag="mm1", name="mm1")
                nc.tensor.matmul(
                    pb[:], cast(ones_row[:]), cast(den_sb[:]), start=True, stop=True
                )
                recip = smpool.tile([P, CHUNK], F32, tag=f"recip{h}", name=f"recip{h}")
                nc.vector.reciprocal(recip[:], pb[:])
                nc.vector.tensor_mul(oTn[h][:], po[h][:], recip[:])
            outproj_chunk(c, oTn)

        def attn_chunk(c, qT):
            # ---- attention for this q-chunk ----
            KB = 4 * c + 5 if c < NCHUNK - 1 else LT
            oTn = [opool.tile([P, CHUNK], MMDT, tag=f"oT{h}", name=f"oT{h}")
                   for h in range(HPC)]
            for h in range(HPC):
                po = ps_acc.tile([P, CHUNK], F32, tag="pvacc", name="pvacc", bufs=2)
                esum = None
                if denom_mode == "esum":
                    # accumulate exp tiles elementwise on DVE; a single
                    # ones-matmul at the end replicates the row sums to all
                    # partitions (replaces one PE stream per kb block)
                    esum = espool.tile([P, CHUNK], MMDT, tag="esum", name="esum")
                    pd = ps_aux.tile([P, CHUNK], F32, tag="aux", name="aux")
                else:
                    pd = ps_aux.tile([P if fused_denb else 1, CHUNK], F32,
                                     tag="aux", name="aux")
                for kb in range(KB):
                    # Diagonal-band blocks (j >= 1): every column below
                    # 128j-1 is fully masked, so restrict all ops to the
                    # valid column range (8B-aligned start). The skipped
                    # region of est is stale but never read.
                    j = kb - 4 * c
                    s0 = 128 * j - 2 if j >= 1 else 0
                    sl = slice(s0, CHUNK)
                    pst = ps_mm.tile([P, CHUNK], F32, tag="mm1", name="mm1")
                    nc.tensor.matmul(
                        pst[:, sl],
                        cast(kT[h][:, kb * P:(kb + 1) * P]),
                        cast(qT[h][:, sl]),
                        start=True,
                        stop=True,
                    )
                    est = estpool.tile([P, CHUNK], MMDT, tag="est", name="est")
                    nc.scalar.activation(est[:, sl], pst[:, sl], AF.Exp, scale=SCALE)
                    if j >= 0:
                        eng = nc.gpsimd if mask_gpsimd else nc.vector
                        eng.tensor_mul(est[:, sl], est[:, sl], masks[j][:, sl])
                    nc.tensor.matmul(
                        po[:, sl],
                        cast(vt[kb][:, h * P:(h + 1) * P]),
                        cast(est[:, sl]),
                        start=(kb == 0),
                        stop=(kb == KB - 1),
                    )
                    if denom_mode == "esum":
                        if kb == 0:
                            nc.vector.tensor_copy(esum[:, sl], est[:, sl])
                        else:
                            nc.vector.tensor_add(
                                esum[:, sl], esum[:, sl], est[:, sl]
                            )
                    else:
                        # row-sum accumulation; fused_denb replicates the sum
                        # to all 128 partitions (ones matrix) so no broadcast
                        # matmul is needed afterwards
                        nc.tensor.matmul(
                            pd[:, sl],
                            cast(ones_mat[:] if fused_denb else ones_col[:]),
                            cast(est[:, sl]),
                            start=(kb == 0),
                            stop=(kb == KB - 1),
                        )
                # normalize: oTn = po * (1 / rowsum) broadcast over partitions
                recip = smpool.tile([P, CHUNK], F32, tag="recip", name="recip")
                if denom_mode == "esum":
                    nc.tensor.matmul(
                        pd[:], cast(ones_mat[:]), cast(esum[:]),
                        start=True, stop=True,
                    )
                    nc.vector.reciprocal(recip[:], pd[:])
                elif fused_denb:
                    nc.vector.reciprocal(recip[:], pd[:])
                else:
                    den_sb = smpool.tile([1, CHUNK], MMDT, tag="den", name="den")
                    if den_on_dve:
                        nc.vector.tensor_copy(den_sb[:], pd[:])
                    else:
                        nc.scalar.copy(den_sb[:], pd[:])
                    if pb_in_mm1:
                        pb = ps_mm.tile([P, CHUNK], F32, tag="mm1", name="mm1")
                    else:
                        pb = ps_aux.tile([P, CHUNK], F32, tag="aux", name="aux")
                    nc.tensor.matmul(
                        pb[:], cast(ones_row[:]), cast(den_sb[:]),
                        start=True, stop=True,
                    )
                    nc.vector.reciprocal(recip[:], pb[:])
                nc.vector.tensor_mul(oTn[h][:], po[:], recip[:])
            outproj_chunk(c, oTn)

        def outproj_chunk(c, oTn):
            # ---- output projection for this chunk ----
            for do in range(DT):
                py = ps_acc.tile([P, CHUNK], F32, tag="acc", name="acc")
                for h in range(HPC):
                    nc.tensor.matmul(
                        py[:],
                        cast(wo[h][:, do * P:(do + 1) * P]),
                        cast(oTn[h][:]),
                        start=(h == 0),
                        stop=(h == HPC - 1),
                    )
                yst = ypool.tile([P, CHUNK], F32, tag="yst", name="yst")
                if yst_on_act:
                    nc.scalar.copy(yst[:], py[:])
                else:
                    nc.vector.tensor_copy(yst[:], py[:])
                nc.sync.dma_start(
                    out=y_d[do * P:(do + 1) * P, c * CHUNK:(c + 1) * CHUNK],
                    in_=yst[:],
                )

        # Pipeline: attention of chunk c needs K/V through block 4c+4, which
        # lives in chunk c+1's rows (the tril(k=1) one-token lookahead). So
        # run projections one chunk ahead of attention. reps>1 repeats the
        # whole compute for benchmarking (amortizes dispatch overhead).
        attn = attn_chunk_ileave if attn_mode == "ileave" else attn_chunk
        for _rep in range(reps):
            qTs = {}
            qTs[0] = proj_chunk(0, xT=xT0 if _rep == 0 else None)
            if attn_order == "small_last":
                # attn(c) only needs proj(c+1); run the smallest chunk (0)
                # last so the un-overlapped kernel tail is as short as
                # possible. Needs qT(0) alive until the end (qpool bufs).
                qTs[1] = proj_chunk(1)
                qTs[2] = proj_chunk(2)
                attn(1, qTs.pop(1))
                qTs[3] = proj_chunk(3)
                attn(2, qTs.pop(2))
                attn(3, qTs.pop(3))
                attn(0, qTs.pop(0))
            else:
                for c in range(1, NCHUNK):
                    qTs[c] = proj_chunk(c)
                    attn(c - 1, qTs.pop(c - 1))
                attn(NCHUNK - 1, qTs.pop(NCHUNK - 1))

    nc.compile()
    return nc


def shard_inputs(x, Wq, bq, Wk, bk, Wv, bv, Wo, bo):
    x = np.asarray(x, dtype=np.float32)
    in_maps = []
    for core in range(NCORES):
        b = core // 2
        g = core % 2
        sl = slice(g * DH, (g + 1) * DH)
        in_maps.append({
            "x": np.ascontiguousarray(x[b].T),
            "wq": np.ascontiguousarray(np.asarray(Wq, np.float32)[:, sl]),
            "wk": np.ascontiguousarray(np.asarray(Wk, np.float32)[:, sl]),
            "wv": np.ascontiguousarray(np.asarray(Wv, np.float32)[:, sl]),
            "wo": np.ascontiguousarray(np.asarray(Wo, np.float32)[sl, :]),
            "bq": np.ascontiguousarray(np.asarray(bq, np.float32)[sl]),
            "bk": np.ascontiguousarray(np.asarray(bk, np.float32)[sl]),
        })
    return in_maps


def unshard_output(results, Wo, bv, bo):
    out = np.empty((B, L, D), dtype=np.float32)
    for b in range(B):
        acc = results[2 * b]["y"] + results[2 * b + 1]["y"]  # [D, L]
        out[b] = acc.T
    corr = np.asarray(bo, np.float32) + np.asarray(bv, np.float32) @ np.asarray(
        Wo, np.float32
    )
    out += corr
    return out


def run(inputs, trace=False, **kw):
    if "nc" not in _cache:
        _cache["nc"] = build_nc()
    nc = _cache["nc"]
    in_maps = shard_inputs(**inputs)
    res = run_bass_kernel_spmd(nc, in_maps, list(range(NCORES)), trace=trace, **kw)
    out = unshard_output(res.results, inputs["Wo"], inputs["bv"], inputs["bo"])
    return out, res


def kernel(**inputs):
    out, _ = run(inputs)
    return out

